# revision 11
# baseline (speedup 1.0000x reference)
"""Trainium2 Bass kernel for nn_DSRLossStateless (DSR loss, stateless).

loss = -sum_t(D_t)/B where D_t comes from an eta-EMA pair (A,B) over
portfolio returns R_t = sum_a w[t,a]*nr[t,a].

Strategy (8 cores, batch-sharded, fp16 inputs, multi-engine split):
  - Host casts both inputs to fp16 (the device pipeline is fp16 anyway:
    the f32 version cast during DMA), halving HBM traffic. Each core
    owns 250k consecutive rows as 125 SBUF partitions x 2000 cols
    (time-major); partition 0 holds the 2000 preceding rows (synthetic
    for core 0) and supplies the scan carry for partition 1.
  - Loads are plain HWDGE DMAs on the sync queue, issued up front.
    Chunks are 250 cols (two 125s first so compute starts early).
  - Software pipelined issue order with a one-chunk lag: DVE's queue is
    [prod_k, L1_k, L4_{k-1}, scans_{k-1}, chains...] so DVE never stalls
    waiting for GpSimd's s2/s3 of the same chunk.
  - Scaled streams: a = A/eta (scan of R), h1 = 1e4*B (scan of 100R^2).
    Chain per range: v1=0.01*h1+R100, v2=a*v1, v3=h1*R,
    negn=0.5*v2-v3 (= -1e4*numer/eta = -numer4), var4=h1-a^2 (=1e4 var),
    rec4=1/var4, srec=sqrt(rec4), qsum += negn*srec*rec4 (= -D_t),
    loss = sum(qsum)/B. var4/v2/v3/t ride GpSimd in the stream; the
    latency-critical tail keeps everything on DVE.
  - Scans are f32, chained per chunk; the partition carry is the
    previous partition's final (c^2000 ~ 1.9e-9 kills older terms),
    applied to cols < CUT as prev = loc + init*c^t.
"""

import sys

sys.path.insert(0, "/opt/trn_rl_repo")

import numpy as np

import concourse.bass as bass
import concourse.bacc as bacc
import concourse.tile as tile
from concourse import mybir
from concourse.bass_utils import run_bass_kernel_spmd
from contextlib import ExitStack

F32 = mybir.dt.float32
F16 = mybir.dt.float16
NF32 = np.float32
NF16 = np.float16

N_CORES = 8
NA = 16            # assets (inner dim)
KP = 126           # SBUF partitions used (0 = prepend/carry-feeder)
L = 2000           # columns (time steps per partition)
LE = L + 1         # local-scan buffer width (col 0 = zero carry)
OWN = (KP - 1) * L      # rows owned per core = 250000
B_TOTAL = N_CORES * OWN # 2000000
CHUNKS = [125, 125] + [250] * 6 + [150, 100]
KCMAX = 250
SCMAX = 750        # max chain FD (the carried range)
ETA = 0.01
EPS = 1e-8
CDEC = NF32(1.0 - ETA)  # 0.99
CUT = 750          # cols >= CUT use the zero-carry chain (init*c^t < fp16 ulp)

AL = mybir.AluOpType
AF = mybir.ActivationFunctionType

_PROGRAM = None


def _build_program():
    nc = bacc.Bacc("TRN2", target_bir_lowering=False, debug=False)

    w_ap = nc.dram_tensor("w", [KP * L, NA], F16, kind="ExternalInput").ap()
    nr_ap = nc.dram_tensor("nr", [KP * L, NA], F16, kind="ExternalInput").ap()
    out_ap = nc.dram_tensor("out", [KP, 8], F32, kind="ExternalOutput").ap()

    # geo[t] = c^t (carry decay for the correction pass); only cols < CUT
    # are ever corrected.
    geo_np = (CDEC ** np.arange(CUT).astype(NF32)).astype(np.float16)
    geo_dram = nc.inline_tensor(
        np.ascontiguousarray(np.broadcast_to(geo_np, (KP, CUT))), name="geoc"
    )

    w_v = w_ap.rearrange("(p t) a -> p (t a)", p=KP)
    nr_v = nr_ap.rearrange("(p t) a -> p (t a)", p=KP)

    with tile.TileContext(nc) as tc, ExitStack() as ctx:
        pers = ctx.enter_context(tc.tile_pool(name="pers", bufs=1))
        loadp = ctx.enter_context(tc.tile_pool(name="load", bufs=4))
        tmpp = ctx.enter_context(tc.tile_pool(name="tmp", bufs=4))

        R16 = pers.tile([KP, L], F16, tag="R16")     # R
        R100 = pers.tile([KP, L], F16, tag="R100")   # 100*R^2
        a16 = pers.tile([KP, L], F16, tag="a16")     # a_prev (= A/eta)
        bp16 = pers.tile([KP, L], F16, tag="bp16")   # h1_prev (= 1e4*B)
        A2s = pers.tile([KP, SCMAX], F16, tag="A2s") # a^2 / v3 scratch
        t16b = pers.tile([KP, SCMAX], F16, tag="t16b")
        t16c = pers.tile([KP, SCMAX], F16, tag="t16c")
        w1 = pers.tile([KP, SCMAX], F32, tag="w1")   # var4 / accum dump
        w2 = pers.tile([KP, SCMAX], F32, tag="w2")   # rec4
        Aloc = pers.tile([KP, LE], F32, tag="Aloc")
        Bloc = pers.tile([KP, LE], F32, tag="Bloc")
        cvec = pers.tile([KP, KCMAX], F32, tag="cvec")
        geoc = pers.tile([KP, CUT], F16, tag="geoc")
        initAB = pers.tile([KP, 2], F32, tag="initAB")
        qsum = pers.tile([KP, 8], F32, tag="qsum")

        # all load DMAs first on the sync HWDGE queue: the stream is the
        # critical resource and sync has nothing else to do
        tiles = []
        off = 0
        for kc in CHUNKS:
            fw = kc * NA
            wt = loadp.tile([KP, KCMAX * NA], F16, tag="wt")
            rt = loadp.tile([KP, KCMAX * NA], F16, tag="rt")
            nc.sync.dma_start(wt[:, 0:fw], w_v[:, off * NA:(off + kc) * NA])
            nc.sync.dma_start(rt[:, 0:fw], nr_v[:, off * NA:(off + kc) * NA])
            tiles.append((off, kc, wt, rt))
            off += kc

        # geoc rides the scalar HWDGE queue up front (tiny, never blocks)
        nc.scalar.dma_start(geoc[:], geo_dram.ap())

        # constants / scan seeds; pin both ACT tables before the stream
        nc.vector.memset(qsum[0:1, 0:2], 1.0)
        nc.scalar.sqrt(qsum[0:1, 0:1], qsum[0:1, 0:1])
        nc.scalar.square(qsum[0:1, 1:2], qsum[0:1, 1:2])
        nc.vector.memset(cvec[:, :], float(CDEC))
        nc.vector.memset(Aloc[:, 0:1], 0.0)
        nc.vector.memset(Bloc[:, 0:1], 0.0)
        nc.vector.memset(initAB[:, :], 0.0)

        def blocks_of(off, kc):
            o = off
            while o < off + kc:
                b = min(250, off + kc - o)
                yield o, b
                o += b

        # stage 1 of a chunk: product + L1 on DVE, s2 + s3 on GpSimd.
        # returns the s3 tiles for the lagged stage 2.
        def stage1(off, kc, wt, rt):
            out = []
            for o, b in blocks_of(off, kc):
                lo = (o - off) * NA
                fw = b * NA
                prod = tmpp.tile([KP, KCMAX * NA], F16, tag="prod")
                s1 = tmpp.tile([KP, KCMAX * 8], F16, tag="s1")
                s2 = tmpp.tile([KP, KCMAX * 4], F16, tag="s2")
                s3 = tmpp.tile([KP, KCMAX * 2], F16, tag="s3")
                nc.vector.tensor_mul(prod[:, 0:fw], wt[:, lo:lo + fw],
                                     rt[:, lo:lo + fw])
                p3 = prod[:, 0:fw].rearrange("p (t a) -> p t a", a=NA)
                s1v = s1[:, 0:b * 8].rearrange("p (t a) -> p t a", a=8)
                s2v = s2[:, 0:b * 4].rearrange("p (t a) -> p t a", a=4)
                s3v = s3[:, 0:b * 2].rearrange("p (t a) -> p t a", a=2)
                nc.vector.tensor_add(s1v[:, :, :], p3[:, :, 0:8], p3[:, :, 8:16])
                nc.gpsimd.tensor_add(s2v[:, :, :], s1v[:, :, 0:4], s1v[:, :, 4:8])
                nc.gpsimd.tensor_add(s3v[:, :, :], s2v[:, :, 0:2], s2v[:, :, 2:4])
                out.append((o, b, s3v))
            return out

        # lagged stage 2: L4 (+R100 on ACT), then the chained scans
        def stage2(off, kc, s3list):
            for o, b, s3v in s3list:
                nc.vector.tensor_add(R16[:, o:o + b], s3v[:, :, 0], s3v[:, :, 1])
                nc.scalar.activation(R100[:, o:o + b], R16[:, o:o + b],
                                     AF.Square, scale=10.0)
            ks = slice(off, off + kc)
            nc.vector.tensor_tensor_scan(
                out=Aloc[:, 1 + off:1 + off + kc], data0=cvec[:, 0:kc],
                data1=R16[:, ks], initial=Aloc[:, off:off + 1],
                op0=AL.mult, op1=AL.add,
            )
            nc.vector.tensor_tensor_scan(
                out=Bloc[:, 1 + off:1 + off + kc], data0=cvec[:, 0:kc],
                data1=R100[:, ks], initial=Bloc[:, off:off + 1],
                op0=AL.mult, op1=AL.add,
            )

        # D-chain for cols [o, o+f): a16/bp16 from ACT casts (nocarry) or
        # the corr pass (carried). var4/rec4 run early so ACT's sqrt
        # overlaps the v1/v2/v3 work. tt picks the engine for the four
        # movable tensor_tensor ops (GpSimd in-stream, DVE in the tail).
        def chain(o, f, qcol, carried=False, tt=None):
            if tt is None:
                tt = nc.gpsimd
            h = slice(o, o + f)
            hv = slice(0, f)
            if not carried:
                nc.scalar.activation(a16[:, h], Aloc[:, o:o + f], AF.Copy)
                nc.scalar.activation(bp16[:, h], Bloc[:, o:o + f], AF.Copy)
            nc.scalar.activation(A2s[:, hv], a16[:, h], AF.Square)
            nc.vector.scalar_tensor_tensor(
                out=t16c[:, hv], in0=bp16[:, h], scalar=0.01, in1=R100[:, h],
                op0=AL.mult, op1=AL.add,
            )                                                       # v1
            tt.tensor_sub(w1[:, hv], bp16[:, h], A2s[:, hv])        # var4 f32
            nc.vector.reciprocal_approx_fast(w2[:, hv], w1[:, hv])  # rec4
            nc.scalar.activation(t16b[:, hv], w2[:, hv], AF.Sqrt)   # srec fp16
            tt.tensor_mul(t16c[:, hv], a16[:, h], t16c[:, hv])      # v2
            tt.tensor_mul(A2s[:, hv], bp16[:, h], R16[:, h])        # v3
            nc.vector.scalar_tensor_tensor(
                out=t16c[:, hv], in0=t16c[:, hv], scalar=0.5, in1=A2s[:, hv],
                op0=AL.mult, op1=AL.subtract,
            )                                                       # negn
            tt.tensor_mul(t16c[:, hv], t16c[:, hv], t16b[:, hv])    # *srec
            nc.vector.scalar_tensor_tensor(
                out=w1[:, hv], in0=t16c[:, hv], scalar=1.0, in1=w2[:, hv],
                op0=AL.mult, op1=AL.mult, accum_out=qsum[:, qcol:qcol + 1],
            )                                                       # -D_t

        # ---- stream, software-pipelined with a one-chunk lag ----
        # chains issue once their columns' scans are in (chunk ends:
        # 125,250,500,750,1000,1250,1500,1750,1900,2000)
        chains_after = {4: (750, 250, 1), 5: (1000, 250, 2),
                        6: (1250, 250, 3), 7: (1500, 250, 4)}
        nchunks = len(tiles)
        s3_prev = None
        for k in range(nchunks + 1):
            if k < nchunks:
                off, kc, wt, rt = tiles[k]
                s3_list = stage1(off, kc, wt, rt)
            if k >= 1:
                joff, jkc, _, _ = tiles[k - 1]
                stage2(joff, jkc, s3_prev)
                if (k - 1) in chains_after:
                    co, cf, qc = chains_after[k - 1]
                    chain(co, cf, qc)
            if k < nchunks:
                s3_prev = s3_list

        # ---- tail ----
        # carries: previous partition's local final (c^2000 kills older terms)
        nc.sync.dma_start(initAB[1:KP, 0:1], Aloc[0:KP - 1, L:LE])
        nc.scalar.dma_start(initAB[1:KP, 1:2], Bloc[0:KP - 1, L:LE])

        # last nocarry range runs while the shift DMAs fly (DVE tt: it is
        # tail-adjacent)
        chain(1750, 250, 5, tt=nc.vector)

        # carried pass over cols [0, CUT), single range, all on DVE
        nc.vector.scalar_tensor_tensor(
            out=a16[:, 0:CUT], in0=geoc[:, 0:CUT], scalar=initAB[:, 0:1],
            in1=Aloc[:, 0:CUT], op0=AL.mult, op1=AL.add,
        )
        nc.vector.scalar_tensor_tensor(
            out=bp16[:, 0:CUT], in0=geoc[:, 0:CUT], scalar=initAB[:, 1:2],
            in1=Bloc[:, 0:CUT], op0=AL.mult, op1=AL.add,
        )
        chain(0, CUT, 0, carried=True, tt=nc.vector)

        # per-partition partials ship out; the host reduces (p0 excluded)
        nc.sync.dma_start(out_ap[:, :], qsum[:, :])

    nc.compile()
    return nc


def _get_program():
    global _PROGRAM
    if _PROGRAM is None:
        _PROGRAM = _build_program()
    return _PROGRAM


def _core0_prepend():
    """2000 synthetic rows encoding the global init (A,B)=(0,EPS).

    All-zero rows leave the scan at (0,0); the last two rows carry returns
    r1, r2 with r2 = -fl16(c*r1) so the A-scan cancels to ~0, while
    eta*(c*r1^2 + r2^2) ~ EPS supplies the B carry.
    """
    w = np.zeros((L, NA), NF16)
    nr = np.zeros((L, NA), NF16)
    c = CDEC
    r1 = NF16(np.sqrt(EPS / (ETA * (float(c) + float(c) ** 2))))
    r2 = NF16(-(c * NF32(r1)))
    w[L - 2, 0] = NF16(1.0)
    nr[L - 2, 0] = r1
    w[L - 1, 0] = NF16(1.0)
    nr[L - 1, 0] = r2
    return w, nr


def _make_in_maps(weights, nr):
    weights = np.asarray(weights, dtype=NF16)
    nr = np.asarray(nr, dtype=NF16)
    pre_w, pre_nr = _core0_prepend()
    in_maps = []
    for m in range(N_CORES):
        s = m * OWN
        if m == 0:
            wm = np.concatenate([pre_w, weights[:OWN]])
            rm = np.concatenate([pre_nr, nr[:OWN]])
        else:
            wm = np.ascontiguousarray(weights[s - L:s + OWN])
            rm = np.ascontiguousarray(nr[s - L:s + OWN])
        in_maps.append({"w": wm, "nr": rm})
    return in_maps


def _run(in_maps, **kwargs):
    nc = _get_program()
    return run_bass_kernel_spmd(nc, in_maps, core_ids=list(range(N_CORES)), **kwargs)


def kernel(weights, next_returns):
    in_maps = _make_in_maps(weights, next_returns)
    res = _run(in_maps)
    total = NF32(0.0)
    for m in range(N_CORES):
        q = np.asarray(res.results[m]["out"], NF32)
        total = NF32(total + np.sum(q[1:, 0:6], dtype=NF32))
    # accum = -D_t summed per partition; loss = -sum(D)/B = total/B
    return NF32(total / NF32(B_TOTAL))


# revision 12
# speedup vs baseline: 1.0183x; 1.0183x over previous
"""Trainium2 Bass kernel for nn_DSRLossStateless (DSR loss, stateless).

loss = -sum_t(D_t)/B where D_t comes from an eta-EMA pair (A,B) over
portfolio returns R_t = sum_a w[t,a]*nr[t,a].

Strategy (8 cores, batch-sharded, fp16 inputs, DVE+ACT only):
  - Host casts both inputs to fp16 (the f32 pipeline cast during DMA
    anyway), halving HBM traffic. Each core owns 250k consecutive rows
    as 125 SBUF partitions x 2000 cols (time-major); partition 0 holds
    the 2000 preceding rows (synthetic for core 0) and supplies the
    scan carry for partition 1.
  - Loads are plain HWDGE DMAs: w rides the sync queue, nr the scalar
    queue, so the two FIFOs' per-DMA completion stalls overlap. Two
    125-col starter chunks let compute begin early; 500-col middle
    chunks keep the stream at full rate.
  - All elementwise work stays on DVE (GpSimd compute contends for the
    same SBUF ports and slows DVE ~2-5x); ACT does the squares, casts,
    sqrt. Per block: product, 4-level pairwise tree -> R (fp16), ACT
    R100=100R^2; per chunk: two chained f32 scans (a = A/eta of R,
    h1 = 1e4*B of R100).
  - Chain per range: v1=0.01*h1+R100, v2=a*v1, v3=h1*R,
    negn=0.5*v2-v3 (= -numer4), var4=h1-a^2 (f32), rec4=1/var4,
    srec=sqrt(rec4) on ACT, qsum += negn*srec*rec4 (= -D_t),
    loss = sum(qsum)/B. var4/rec4 are issued first so ACT's sqrt
    hides under the v1/v2/v3 work.
  - The partition carry is the previous partition's final scan state
    (c^2000 ~ 1.9e-9 kills older terms), applied to cols < CUT as
    prev = loc + init*c^t in one 750-col corrected pass in the tail.
"""

import sys

sys.path.insert(0, "/opt/trn_rl_repo")

import numpy as np

import concourse.bass as bass
import concourse.bacc as bacc
import concourse.tile as tile
from concourse import mybir
from concourse.bass_utils import run_bass_kernel_spmd
from contextlib import ExitStack

F32 = mybir.dt.float32
F16 = mybir.dt.float16
NF32 = np.float32
NF16 = np.float16

N_CORES = 8
NA = 16            # assets (inner dim)
KP = 126           # SBUF partitions used (0 = prepend/carry-feeder)
L = 2000           # columns (time steps per partition)
LE = L + 1         # local-scan buffer width (col 0 = zero carry)
OWN = (KP - 1) * L      # rows owned per core = 250000
B_TOTAL = N_CORES * OWN # 2000000
CHUNKS = [125, 125, 250, 500, 500, 250, 150, 100]
KCMAX = 500
BLK = 250
SCMAX = 750        # max chain FD (the carried range)
ETA = 0.01
EPS = 1e-8
CDEC = NF32(1.0 - ETA)  # 0.99
CUT = 750          # cols >= CUT use the zero-carry chain (init*c^t < fp16 ulp)

AL = mybir.AluOpType
AF = mybir.ActivationFunctionType

_PROGRAM = None


def _build_program():
    nc = bacc.Bacc("TRN2", target_bir_lowering=False, debug=False)

    w_ap = nc.dram_tensor("w", [KP * L, NA], F16, kind="ExternalInput").ap()
    nr_ap = nc.dram_tensor("nr", [KP * L, NA], F16, kind="ExternalInput").ap()
    out_ap = nc.dram_tensor("out", [KP, 8], F32, kind="ExternalOutput").ap()

    # geo[t] = c^t (carry decay for the correction pass); only cols < CUT
    # are ever corrected.
    geo_np = (CDEC ** np.arange(CUT).astype(NF32)).astype(np.float16)
    geo_dram = nc.inline_tensor(
        np.ascontiguousarray(np.broadcast_to(geo_np, (KP, CUT))), name="geoc"
    )

    w_v = w_ap.rearrange("(p t) a -> p (t a)", p=KP)
    nr_v = nr_ap.rearrange("(p t) a -> p (t a)", p=KP)

    with tile.TileContext(nc) as tc, ExitStack() as ctx:
        pers = ctx.enter_context(tc.tile_pool(name="pers", bufs=1))
        loadp = ctx.enter_context(tc.tile_pool(name="load", bufs=3))
        tmpp = ctx.enter_context(tc.tile_pool(name="tmp", bufs=2))

        R16 = pers.tile([KP, L], F16, tag="R16")     # R
        R100 = pers.tile([KP, L], F16, tag="R100")   # 100*R^2
        a16 = pers.tile([KP, L], F16, tag="a16")     # a_prev (= A/eta)
        bp16 = pers.tile([KP, L], F16, tag="bp16")   # h1_prev (= 1e4*B)
        A2s = pers.tile([KP, SCMAX], F16, tag="A2s") # a^2 / v3 scratch
        t16b = pers.tile([KP, SCMAX], F16, tag="t16b")
        t16c = pers.tile([KP, SCMAX], F16, tag="t16c")
        w1 = pers.tile([KP, SCMAX], F32, tag="w1")   # var4 / accum dump
        w2 = pers.tile([KP, SCMAX], F32, tag="w2")   # rec4
        Aloc = pers.tile([KP, LE], F32, tag="Aloc")
        Bloc = pers.tile([KP, LE], F32, tag="Bloc")
        cvec = pers.tile([KP, KCMAX], F32, tag="cvec")
        geoc = pers.tile([KP, CUT], F16, tag="geoc")
        initAB = pers.tile([KP, 2], F32, tag="initAB")
        qsum = pers.tile([KP, 8], F32, tag="qsum")

        # all load DMAs first: w on the sync HWDGE queue, nr on the
        # scalar HWDGE queue (two FIFOs overlap their completion stalls)
        tiles = []
        off = 0
        for kc in CHUNKS:
            fw = kc * NA
            wt = loadp.tile([KP, KCMAX * NA], F16, tag="wt")
            rt = loadp.tile([KP, KCMAX * NA], F16, tag="rt")
            nc.sync.dma_start(wt[:, 0:fw], w_v[:, off * NA:(off + kc) * NA])
            nc.scalar.dma_start(rt[:, 0:fw], nr_v[:, off * NA:(off + kc) * NA])
            tiles.append((off, kc, wt, rt))
            off += kc

        # geoc rides the sync queue after the w loads (tiny, arrives long
        # before the tail needs it)
        nc.sync.dma_start(geoc[:], geo_dram.ap())

        # constants / scan seeds; pin both ACT tables before the stream
        nc.vector.memset(qsum[0:1, 0:2], 1.0)
        nc.scalar.sqrt(qsum[0:1, 0:1], qsum[0:1, 0:1])
        nc.scalar.square(qsum[0:1, 1:2], qsum[0:1, 1:2])
        nc.vector.memset(cvec[:, :], float(CDEC))
        nc.vector.memset(Aloc[:, 0:1], 0.0)
        nc.vector.memset(Bloc[:, 0:1], 0.0)
        nc.vector.memset(initAB[:, :], 0.0)

        def blocks_of(off, kc):
            o = off
            while o < off + kc:
                b = min(BLK, off + kc - o)
                yield o, b
                o += b

        def do_chunk(off, kc, wt, rt):
            for o, b in blocks_of(off, kc):
                lo = (o - off) * NA
                fw = b * NA
                prod = tmpp.tile([KP, BLK * NA], F16, tag="prod")
                s1 = tmpp.tile([KP, BLK * 8], F16, tag="s1")
                s2 = tmpp.tile([KP, BLK * 4], F16, tag="s2")
                s3 = tmpp.tile([KP, BLK * 2], F16, tag="s3")
                nc.vector.tensor_mul(prod[:, 0:fw], wt[:, lo:lo + fw],
                                     rt[:, lo:lo + fw])
                p3 = prod[:, 0:fw].rearrange("p (t a) -> p t a", a=NA)
                s1v = s1[:, 0:b * 8].rearrange("p (t a) -> p t a", a=8)
                s2v = s2[:, 0:b * 4].rearrange("p (t a) -> p t a", a=4)
                s3v = s3[:, 0:b * 2].rearrange("p (t a) -> p t a", a=2)
                nc.vector.tensor_add(s1v[:, :, :], p3[:, :, 0:8], p3[:, :, 8:16])
                nc.vector.tensor_add(s2v[:, :, :], s1v[:, :, 0:4], s1v[:, :, 4:8])
                nc.vector.tensor_add(s3v[:, :, :], s2v[:, :, 0:2], s2v[:, :, 2:4])
                nc.vector.tensor_add(R16[:, o:o + b], s3v[:, :, 0], s3v[:, :, 1])
                nc.scalar.activation(R100[:, o:o + b], R16[:, o:o + b],
                                     AF.Square, scale=10.0)
            ks = slice(off, off + kc)
            nc.vector.tensor_tensor_scan(
                out=Aloc[:, 1 + off:1 + off + kc], data0=cvec[:, 0:kc],
                data1=R16[:, ks], initial=Aloc[:, off:off + 1],
                op0=AL.mult, op1=AL.add,
            )
            nc.vector.tensor_tensor_scan(
                out=Bloc[:, 1 + off:1 + off + kc], data0=cvec[:, 0:kc],
                data1=R100[:, ks], initial=Bloc[:, off:off + 1],
                op0=AL.mult, op1=AL.add,
            )

        # D-chain for cols [o, o+f): a16/bp16 from ACT casts (nocarry) or
        # the corr pass (carried). var4/rec4 run early so ACT's sqrt
        # overlaps the v1/v2/v3 work.
        def chain(o, f, qcol, carried=False):
            h = slice(o, o + f)
            hv = slice(0, f)
            if not carried:
                nc.scalar.activation(a16[:, h], Aloc[:, o:o + f], AF.Copy)
                nc.scalar.activation(bp16[:, h], Bloc[:, o:o + f], AF.Copy)
            nc.scalar.activation(A2s[:, hv], a16[:, h], AF.Square)
            nc.vector.tensor_sub(w1[:, hv], bp16[:, h], A2s[:, hv])    # var4
            nc.vector.reciprocal_approx_fast(w2[:, hv], w1[:, hv])     # rec4
            nc.scalar.activation(t16b[:, hv], w2[:, hv], AF.Sqrt)      # srec
            nc.vector.scalar_tensor_tensor(
                out=t16c[:, hv], in0=bp16[:, h], scalar=0.01, in1=R100[:, h],
                op0=AL.mult, op1=AL.add,
            )                                                          # v1
            nc.vector.tensor_mul(t16c[:, hv], a16[:, h], t16c[:, hv])  # v2
            nc.vector.tensor_mul(A2s[:, hv], bp16[:, h], R16[:, h])    # v3
            nc.vector.scalar_tensor_tensor(
                out=t16c[:, hv], in0=t16c[:, hv], scalar=0.5, in1=A2s[:, hv],
                op0=AL.mult, op1=AL.subtract,
            )                                                          # negn
            nc.vector.tensor_mul(t16c[:, hv], t16c[:, hv], t16b[:, hv])  # *srec
            nc.vector.scalar_tensor_tensor(
                out=w1[:, hv], in0=t16c[:, hv], scalar=1.0, in1=w2[:, hv],
                op0=AL.mult, op1=AL.mult, accum_out=qsum[:, qcol:qcol + 1],
            )                                                          # -D_t

        # ---- stream ----
        # chunk ends: 125,250,500,1000,1500,1750,1900,2000
        chains_after = {4: (750, 750, 1), 5: (1500, 250, 2)}
        for ci, (off, kc, wt, rt) in enumerate(tiles):
            do_chunk(off, kc, wt, rt)
            if ci in chains_after:
                co, cf, qc = chains_after[ci]
                chain(co, cf, qc)

        # ---- tail ----
        # carries: previous partition's local final (c^2000 kills older terms)
        nc.sync.dma_start(initAB[1:KP, 0:1], Aloc[0:KP - 1, L:LE])
        nc.scalar.dma_start(initAB[1:KP, 1:2], Bloc[0:KP - 1, L:LE])

        # last nocarry range runs while the shift DMAs fly
        chain(1750, 250, 3)

        # carried pass over cols [0, CUT), single range
        nc.vector.scalar_tensor_tensor(
            out=a16[:, 0:CUT], in0=geoc[:, 0:CUT], scalar=initAB[:, 0:1],
            in1=Aloc[:, 0:CUT], op0=AL.mult, op1=AL.add,
        )
        nc.vector.scalar_tensor_tensor(
            out=bp16[:, 0:CUT], in0=geoc[:, 0:CUT], scalar=initAB[:, 1:2],
            in1=Bloc[:, 0:CUT], op0=AL.mult, op1=AL.add,
        )
        chain(0, CUT, 0, carried=True)

        # per-partition partials ship out; the host reduces (p0 excluded)
        nc.sync.dma_start(out_ap[:, :], qsum[:, :])

    nc.compile()
    return nc


def _get_program():
    global _PROGRAM
    if _PROGRAM is None:
        _PROGRAM = _build_program()
    return _PROGRAM


def _core0_prepend():
    """2000 synthetic rows encoding the global init (A,B)=(0,EPS).

    All-zero rows leave the scan at (0,0); the last two rows carry returns
    r1, r2 with r2 = -fl16(c*r1) so the A-scan cancels to ~0, while
    eta*(c*r1^2 + r2^2) ~ EPS supplies the B carry.
    """
    w = np.zeros((L, NA), NF16)
    nr = np.zeros((L, NA), NF16)
    c = CDEC
    r1 = NF16(np.sqrt(EPS / (ETA * (float(c) + float(c) ** 2))))
    r2 = NF16(-(c * NF32(r1)))
    w[L - 2, 0] = NF16(1.0)
    nr[L - 2, 0] = r1
    w[L - 1, 0] = NF16(1.0)
    nr[L - 1, 0] = r2
    return w, nr


def _make_in_maps(weights, nr):
    weights = np.asarray(weights, dtype=NF16)
    nr = np.asarray(nr, dtype=NF16)
    pre_w, pre_nr = _core0_prepend()
    in_maps = []
    for m in range(N_CORES):
        s = m * OWN
        if m == 0:
            wm = np.concatenate([pre_w, weights[:OWN]])
            rm = np.concatenate([pre_nr, nr[:OWN]])
        else:
            wm = np.ascontiguousarray(weights[s - L:s + OWN])
            rm = np.ascontiguousarray(nr[s - L:s + OWN])
        in_maps.append({"w": wm, "nr": rm})
    return in_maps


def _run(in_maps, **kwargs):
    nc = _get_program()
    return run_bass_kernel_spmd(nc, in_maps, core_ids=list(range(N_CORES)), **kwargs)


def kernel(weights, next_returns):
    in_maps = _make_in_maps(weights, next_returns)
    res = _run(in_maps)
    total = NF32(0.0)
    for m in range(N_CORES):
        q = np.asarray(res.results[m]["out"], NF32)
        total = NF32(total + np.sum(q[1:, 0:4], dtype=NF32))
    # accum = -D_t summed per partition; loss = -sum(D)/B = total/B
    return NF32(total / NF32(B_TOTAL))
